# revision 1
# baseline (speedup 1.0000x reference)
"""ChannelAttentionModule Trainium2 kernel.

Reference computation (B=128, C=704, L=1024, G=11 groups of GW=64 channels):
    y_avg = mean(x, -1); y_max = max(x, -1)                      # [B, C]
    gate  = sigmoid(mlp(y_avg) + mlp(y_max))                     # [B, C]
    out   = x * gate[:, :, None]
where mlp is a per-group linear pair (W1[g]: 64x16, W2[g]: 16x64) with NO
nonlinearity between them, so mlp(a) + mlp(b) = a @ Wc + b @ Wc with
Wc[g] = W1[g] @ W2[g] (64x64), and mean = sum/L can be folded into a
pre-scaled copy of Wc.

Sharding: data-parallel on batch across 8 cores (16 batches/core). Two
consecutive batches = 2*704 = 1408 rows = exactly 11 tiles of 128 rows, and
each 64-row half-tile is one complete (batch, group) channel block, so every
[128, 1024] tile's gate depends only on that tile's own row stats:
    load 8 tiles per 4MB DMA -> per tile: reduce_sum + reduce_max + combine
    (DVE) -> one matmul against a 128x128 block-diagonal combined weight
    (PE) -> psum->sbuf copy (DVE) -> sigmoid (ACT) -> per-partition scaled
    in-place copy (ACT) -> store 4MB.
Best measured: ~286 us/core (HBM roofline ~258 us at 358 GB/s/core).
"""

import os
import sys

import numpy as np

for _p in ("/opt/trn_rl_repo", "/root/.axon_site/_ro/trn_rl_repo"):
    if os.path.isdir(_p) and _p not in sys.path:
        sys.path.insert(0, _p)

import concourse.bacc as bacc
import concourse.bass as bass
import concourse.tile as tile
from concourse import mybir
from concourse.bass_utils import run_bass_kernel_spmd

B, C, L = 128, 704, 1024
G, GW = 11, 64
NCORES = 8
BPC = B // NCORES            # batches per core = 16
NPAIRS = BPC // 2            # 8
PAIR_ROWS = 2 * C            # 1408
NTILES = PAIR_ROWS // 128    # 11
ROWS = BPC * C               # 11264
F32 = mybir.dt.float32

_PROGRAM = None


def _build_program(
    npairs=NPAIRS, blk=8, xbufs=4, sbufs=16, act_num=0, act_den=5, dve_own=True
):
    # blk row-tiles ride in each DMA (blk*512KB transfers) to amortize DMA
    # fixed cost. Per 128-row subtile: reduce_max (DVE) + reduce_sum (DVE,
    # or ACT via activation-with-accum for act_num/act_den of subtiles to
    # balance engine load) -> combine (DVE) -> matmul vs block-diag weight
    # (PE) -> sigmoid straight from PSUM (ACT) -> scaled in-place copy
    # (ACT) -> store.
    nc = bacc.Bacc(None)
    rows = npairs * PAIR_ROWS
    ntile = rows // 128
    assert ntile % blk == 0
    x = nc.declare_dram_parameter("x", [rows, L], F32, isOutput=False)
    w = nc.declare_dram_parameter("W", [128, NTILES * 128], F32, isOutput=False)
    out = nc.declare_dram_parameter("out", [rows, L], F32, isOutput=True)
    xr = x[:, :].rearrange("(n a p) l -> n p a l", a=blk, p=128)
    outr = out[:, :].rearrange("(n a p) l -> n p a l", a=blk, p=128)

    with tile.TileContext(nc) as tc:
        with (
            tc.tile_pool(name="singles", bufs=1) as singles,
            tc.tile_pool(name="xp", bufs=xbufs) as xp,
            tc.tile_pool(name="small", bufs=sbufs) as small,
            tc.tile_pool(name="junkp", bufs=2) as junkp,
            tc.tile_pool(name="psum", bufs=8, space=bass.MemorySpace.PSUM) as psums,
        ):
            if dve_own:
                wt_raw = singles.tile([128, NTILES * 128], F32)
                nc.sync.dma_start(out=wt_raw, in_=w[:, :])
                wt = singles.tile([128, NTILES * 128], F32)
                nc.vector.tensor_copy(out=wt, in_=wt_raw)
            else:
                wt = singles.tile([128, NTILES * 128], F32)
                nc.sync.dma_start(out=wt, in_=w[:, :])

            for n in range(ntile // blk):
                xt = xp.tile([128, blk, L], F32)
                nc.sync.dma_start(out=xt, in_=xr[n])
                for a in range(blk):
                    gi = n * blk + a
                    t = gi % NTILES
                    xs = xt[:, a, :]
                    s = small.tile([128, 1], F32, tag="s")
                    m = small.tile([128, 1], F32, tag="m")
                    if (gi * act_num) % act_den < act_num:
                        junk = junkp.tile([128, L], F32, tag="j")
                        nc.scalar.activation(
                            out=junk, in_=xs,
                            func=mybir.ActivationFunctionType.Copy,
                            accum_out=s,
                        )
                    else:
                        nc.vector.reduce_sum(out=s, in_=xs, axis=mybir.AxisListType.X)
                    nc.vector.reduce_max(out=m, in_=xs, axis=mybir.AxisListType.X)
                    comb = small.tile([128, 1], F32, tag="c")
                    nc.vector.tensor_scalar(
                        out=comb, in0=s, scalar1=1.0 / L, scalar2=m,
                        op0=mybir.AluOpType.mult, op1=mybir.AluOpType.add,
                    )

                    pc = psums.tile([128, 1], F32)
                    nc.tensor.matmul(
                        pc, wt[:, t * 128 : (t + 1) * 128], comb,
                        start=True, stop=True,
                    )
                    if dve_own:
                        gsb = small.tile([128, 1], F32, tag="o")
                        nc.vector.tensor_copy(out=gsb, in_=pc)
                        sig_in = gsb
                    else:
                        sig_in = pc
                    gate = small.tile([128, 1], F32, tag="g")
                    nc.scalar.activation(
                        out=gate, in_=sig_in, func=mybir.ActivationFunctionType.Sigmoid
                    )
                    nc.scalar.mul(out=xs, in_=xs, mul=gate)
                nc.sync.dma_start(out=outr[n], in_=xt)
    if not nc.is_finalized():
        nc.finalize()
    return nc


def _build_program_contig(npairs=NPAIRS, xbufs=4, sbufs=16):
    # Contiguous-HBM layout: each DMA block is 512 consecutive rows and
    # partition p holds rows [r0+4p, r0+4p+4) -- 16KB of contiguous DRAM per
    # partition (128KB per SDMA engine). Slice a of the [128, 4096] tile is
    # row r0+4p+a, so a group's 64 channels span 16 partitions x 4 slices;
    # the MLP contracts over all 4 slices with per-(phase, a_in, a_out)
    # permuted block-diagonal weights (phase = block % 11: 512*11 = 0 mod
    # 704), 4 accumulating matmuls per output slice.
    nc = bacc.Bacc(None)
    rows = npairs * PAIR_ROWS
    nblocks = rows // 512
    assert rows % 512 == 0
    wcols = NTILES * 16 * 128
    x = nc.declare_dram_parameter("x", [rows, L], F32, isOutput=False)
    w = nc.declare_dram_parameter("W", [128, wcols], F32, isOutput=False)
    out = nc.declare_dram_parameter("out", [rows, L], F32, isOutput=True)
    xr = x[:, :].rearrange("(n p a) l -> n p (a l)", p=128, a=4)
    outr = out[:, :].rearrange("(n p a) l -> n p (a l)", p=128, a=4)

    with tile.TileContext(nc) as tc:
        with (
            tc.tile_pool(name="singles", bufs=1) as singles,
            tc.tile_pool(name="xp", bufs=xbufs) as xp,
            tc.tile_pool(name="small", bufs=sbufs) as small,
            tc.tile_pool(name="psum", bufs=8, space=bass.MemorySpace.PSUM) as psums,
        ):
            wt = singles.tile([128, wcols], F32)
            nc.sync.dma_start(out=wt, in_=w[:, :])

            for n in range(nblocks):
                ph = n % NTILES
                xt = xp.tile([128, 4 * L], F32)
                nc.sync.dma_start(out=xt, in_=xr[n])
                combs = []
                for a in range(4):
                    xs = xt[:, a * L : (a + 1) * L]
                    s = small.tile([128, 1], F32, tag="s")
                    m = small.tile([128, 1], F32, tag="m")
                    nc.vector.reduce_sum(out=s, in_=xs, axis=mybir.AxisListType.X)
                    nc.vector.reduce_max(out=m, in_=xs, axis=mybir.AxisListType.X)
                    comb = small.tile([128, 1], F32, tag=f"c{a}")
                    nc.vector.tensor_scalar(
                        out=comb, in0=s, scalar1=1.0 / L, scalar2=m,
                        op0=mybir.AluOpType.mult, op1=mybir.AluOpType.add,
                    )
                    combs.append(comb)

                for a_out in range(4):
                    pc = psums.tile([128, 1], F32)
                    for a_in in range(4):
                        j = (ph * 16 + a_in * 4 + a_out) * 128
                        nc.tensor.matmul(
                            pc, wt[:, j : j + 128], combs[a_in],
                            start=(a_in == 0), stop=(a_in == 3),
                        )
                    gsb = small.tile([128, 1], F32, tag="o")
                    nc.vector.tensor_copy(out=gsb, in_=pc)
                    gate = small.tile([128, 1], F32, tag="g")
                    nc.scalar.activation(
                        out=gate, in_=gsb, func=mybir.ActivationFunctionType.Sigmoid
                    )
                    nc.scalar.mul(
                        out=xt[:, a_out * L : (a_out + 1) * L],
                        in_=xt[:, a_out * L : (a_out + 1) * L],
                        mul=gate,
                    )
                nc.sync.dma_start(out=outr[n], in_=xt)
    if not nc.is_finalized():
        nc.finalize()
    return nc


def _pack_weights_contig(W1, W2):
    # Wtab[k, ((ph*4 + a_in)*4 + a_out)*128 + m] = Wc[g][c_in%64, c_out%64]
    # where c_in = (ph*512 + 4k + a_in) % 704, c_out = (ph*512 + 4m + a_out)
    # % 704, nonzero only when c_in and c_out share a group AND the same
    # batch row pair-half (rows of one batch stay within 704-row spans, and
    # groups never straddle the mod-704 wrap since 704 = 11*64).
    Wc = np.einsum(
        "gch,ghd->gcd", W1.astype(np.float64), W2.astype(np.float64)
    ).astype(np.float32)
    idx = np.arange(128)
    wtab = np.zeros((128, NTILES * 16, 128), np.float32)
    for ph in range(NTILES):
        base = ph * 512
        for a_in in range(4):
            r_in = base + 4 * idx + a_in          # absolute row in pair
            for a_out in range(4):
                r_out = base + 4 * idx + a_out
                same_b = (r_in[:, None] // C) == (r_out[None, :] // C)
                c_in, c_out = r_in % C, r_out % C
                same_g = (c_in[:, None] // GW) == (c_out[None, :] // GW)
                mat = np.where(
                    same_b & same_g,
                    Wc[(c_in // GW)[:, None], (c_in % GW)[:, None], (c_out % GW)[None, :]],
                    0.0,
                )
                wtab[:, ph * 16 + a_in * 4 + a_out, :] = mat
    return wtab.reshape(128, NTILES * 16 * 128)


def _pack_weights(W1, W2):
    # Wc[g] = W1[g] @ W2[g]; tile t holds blocks 2t (partitions 0:64) and
    # 2t+1 (partitions 64:128); block k -> group k % 11. The 1/L mean scale
    # is applied on DVE when combining sum+max, so weights are unscaled.
    Wc = np.einsum(
        "gch,ghd->gcd", W1.astype(np.float64), W2.astype(np.float64)
    ).astype(np.float32)
    wpk = np.zeros((128, NTILES, 128), np.float32)
    for t in range(NTILES):
        gt, gb = (2 * t) % G, (2 * t + 1) % G
        wpk[0:64, t, 0:64] = Wc[gt]
        wpk[64:128, t, 64:128] = Wc[gb]
    return wpk.reshape(128, NTILES * 128)


def _get_program():
    global _PROGRAM
    if _PROGRAM is None:
        _PROGRAM = _build_program()
    return _PROGRAM


_PACK = None


def run(x, W1, W2, trace=False, **kwargs):
    nc = _get_program()
    pack = _PACK if _PACK is not None else _pack_weights
    wpk = pack(np.asarray(W1), np.asarray(W2))
    xs = np.ascontiguousarray(x).reshape(NCORES, ROWS, L)
    in_maps = [{"x": xs[i], "W": wpk} for i in range(NCORES)]
    res = run_bass_kernel_spmd(
        nc, in_maps, core_ids=list(range(NCORES)), trace=trace, **kwargs
    )
    out = np.empty((NCORES, ROWS, L), np.float32)
    for i in range(NCORES):
        out[i] = res.results[i]["out"]
    return out.reshape(B, C, L), res


def kernel(x, W1, W2):
    out, _ = run(x, W1, W2)
    return out



# revision 2
# speedup vs baseline: 1.3261x; 1.3261x over previous
"""ChannelAttentionModule Trainium2 kernel.

Reference computation (B=128, C=704, L=1024, G=11 groups of GW=64 channels):
    y_avg = mean(x, -1); y_max = max(x, -1)                      # [B, C]
    gate  = sigmoid(mlp(y_avg) + mlp(y_max))                     # [B, C]
    out   = x * gate[:, :, None]
where mlp is a per-group linear pair (W1[g]: 64x16, W2[g]: 16x64) with NO
nonlinearity between them, so mlp(a) + mlp(b) = a @ Wc + b @ Wc with
Wc[g] = W1[g] @ W2[g] (64x64), and mean = sum/L is folded in on DVE.

Sharding: data-parallel on batch across 8 cores (16 batches/core). Two
consecutive batches = 2*704 = 1408 rows = 11 row-tiles of 128, and each
64-row half-tile is one complete (batch, group) channel block. Blocks are
gathered PHASE-major: block t holds row-tile phase t (of 11) from all 8
batch-pairs, so every [128, 8, 1024] block shares ONE 128x128 block-diagonal
weight:
    load 4MB block -> reduce_sum + reduce_max over the whole block (2 DVE
    instrs) -> combine s/L+m (DVE) -> one matmul [128,128]x[128,8] (PE) ->
    sigmoid from PSUM (ACT) -> 8 per-pair scaled copies to an fp16 tile
    (ACT) -> store 2MB fp16.
fp16 stores halve write traffic (output quantization ~3e-4 rel, gate 2e-2);
the host upcasts to fp32. The last block is split into 4 sub-blocks to
shorten the pipeline drain.
"""

import os
import sys

import numpy as np

for _p in ("/opt/trn_rl_repo", "/root/.axon_site/_ro/trn_rl_repo"):
    if os.path.isdir(_p) and _p not in sys.path:
        sys.path.insert(0, _p)

import concourse.bacc as bacc
import concourse.bass as bass
import concourse.tile as tile
from concourse import mybir
from concourse.bass_utils import run_bass_kernel_spmd

B, C, L = 128, 704, 1024
G, GW = 11, 64
NCORES = 8
BPC = B // NCORES            # batches per core = 16
NPAIRS = BPC // 2            # 8
PAIR_ROWS = 2 * C            # 1408
NTILES = PAIR_ROWS // 128    # 11
ROWS = BPC * C               # 11264
F32 = mybir.dt.float32
F16 = mybir.dt.float16

_PROGRAM = None


def _build_program_v3(
    npairs=NPAIRS,
    xbufs=4,
    obufs=3,
    sbufs=8,
    split=1,
    last_split=4,
    store_q="sync",
    out16=True,
    dve_own=True,
    sig_psum=True,
):
    # Phase-major blocks: block t = [128, npairs, L] where slice a is
    # row-tile phase t of batch-pair a; all slices share weight t. The last
    # block is processed in `last_split` sub-blocks so the final
    # load->reduce->gate->scale->store chain (the pipeline drain) is short.
    nc = bacc.Bacc(None)
    rows = npairs * PAIR_ROWS
    odt = F16 if out16 else F32
    x = nc.declare_dram_parameter("x", [rows, L], F32, isOutput=False)
    w = nc.declare_dram_parameter("W", [128, NTILES * 128], F32, isOutput=False)
    out = nc.declare_dram_parameter("out", [rows, L], odt, isOutput=True)
    xr = x[:, :].rearrange("(a t p) l -> t p a l", a=npairs, t=NTILES, p=128)
    outr = out[:, :].rearrange("(a t p) l -> t p a l", a=npairs, t=NTILES, p=128)

    def subs_for(t):
        k = last_split if t == NTILES - 1 else split
        base = npairs // k
        return [(i * base, base) for i in range(k)]

    with tile.TileContext(nc) as tc:
        with (
            tc.tile_pool(name="singles", bufs=1) as singles,
            tc.tile_pool(name="xp", bufs=xbufs) as xp,
            tc.tile_pool(name="op", bufs=obufs) as op,
            tc.tile_pool(name="small", bufs=sbufs) as small,
            tc.tile_pool(name="psum", bufs=8, space=bass.MemorySpace.PSUM) as psums,
        ):
            if dve_own:
                wt_raw = singles.tile([128, NTILES * 128], F32)
                nc.sync.dma_start(out=wt_raw, in_=w[:, :])
                wt = singles.tile([128, NTILES * 128], F32)
                nc.vector.tensor_copy(out=wt, in_=wt_raw)
            else:
                wt = singles.tile([128, NTILES * 128], F32)
                nc.sync.dma_start(out=wt, in_=w[:, :])

            for t in range(NTILES):
                pieces = subs_for(t)
                xt = xp.tile([128, npairs, L], F32, tag="x")
                for a0, cnt in pieces:
                    nc.sync.dma_start(
                        out=xt[:, a0 : a0 + cnt, :], in_=xr[t][:, a0 : a0 + cnt, :]
                    )
                for a0, cnt in pieces:
                    xs = xt[:, a0 : a0 + cnt, :]
                    s = small.tile([128, cnt], F32, tag="s")
                    m = small.tile([128, cnt], F32, tag="m")
                    nc.vector.reduce_sum(out=s, in_=xs, axis=mybir.AxisListType.X)
                    nc.vector.reduce_max(out=m, in_=xs, axis=mybir.AxisListType.X)
                    comb = small.tile([128, cnt], F32, tag="c")
                    nc.vector.scalar_tensor_tensor(
                        out=comb, in0=s, scalar=1.0 / L, in1=m,
                        op0=mybir.AluOpType.mult, op1=mybir.AluOpType.add,
                    )
                    pc = psums.tile([128, cnt], F32)
                    nc.tensor.matmul(
                        pc, wt[:, t * 128 : (t + 1) * 128], comb,
                        start=True, stop=True,
                    )
                    if sig_psum:
                        sig_in = pc
                    else:
                        gsb = small.tile([128, cnt], F32, tag="gs")
                        nc.vector.tensor_copy(out=gsb, in_=pc)
                        sig_in = gsb
                    gate = small.tile([128, cnt], F32, tag="g")
                    nc.scalar.activation(
                        out=gate, in_=sig_in, func=mybir.ActivationFunctionType.Sigmoid
                    )
                    ot = op.tile([128, cnt, L], odt, tag="o")
                    for a in range(cnt):
                        nc.scalar.mul(
                            out=ot[:, a, :], in_=xs[:, a, :], mul=gate[:, a : a + 1]
                        )
                    seng = nc.scalar if store_q == "act" else nc.sync
                    seng.dma_start(
                        out=outr[t][:, a0 : a0 + cnt, :], in_=ot[:, :cnt, :]
                    )
    if not nc.is_finalized():
        nc.finalize()
    return nc


def _pack_weights(W1, W2):
    # Wc[g] = W1[g] @ W2[g]; phase t holds channel blocks 2t (partitions
    # 0:64) and 2t+1 (partitions 64:128); block k -> group k % 11. The 1/L
    # mean scale is applied on DVE when combining sum+max, so weights are
    # unscaled.
    Wc = np.einsum(
        "gch,ghd->gcd", W1.astype(np.float64), W2.astype(np.float64)
    ).astype(np.float32)
    wpk = np.zeros((128, NTILES, 128), np.float32)
    for t in range(NTILES):
        gt, gb = (2 * t) % G, (2 * t + 1) % G
        wpk[0:64, t, 0:64] = Wc[gt]
        wpk[64:128, t, 64:128] = Wc[gb]
    return wpk.reshape(128, NTILES * 128)


def _get_program():
    global _PROGRAM
    if _PROGRAM is None:
        _PROGRAM = _build_program_v3()
    return _PROGRAM


_PACK = None


def run(x, W1, W2, trace=False, **kwargs):
    nc = _get_program()
    pack = _PACK if _PACK is not None else _pack_weights
    wpk = pack(np.asarray(W1), np.asarray(W2))
    xs = np.ascontiguousarray(x).reshape(NCORES, ROWS, L)
    in_maps = [{"x": xs[i], "W": wpk} for i in range(NCORES)]
    res = run_bass_kernel_spmd(
        nc, in_maps, core_ids=list(range(NCORES)), trace=trace, **kwargs
    )
    out = np.empty((NCORES, ROWS, L), np.float32)
    for i in range(NCORES):
        out[i] = res.results[i]["out"]
    return out.reshape(B, C, L), res


def kernel(x, W1, W2):
    out, _ = run(x, W1, W2)
    return out
